# revision 50
# baseline (speedup 1.0000x reference)
"""Trainium2 Bass kernel for the dense_transformer problem.

Data-parallel over batch: 8 NeuronCores x (B/8) sequences each.

v2: fp8 (e4m3) DoubleRow matmuls everywhere big, x16 value domain for
features/weights (scale factors fold into activation scale slots and the
tanh-affine constants), tanh-form sigmoids so the whole kernel uses one
activation table set (exp_and_others), ce/ceT built without PE outer
products, hrpT computed directly in transposed orientation (no PE
transposes for HRP).

Self-contained: only imports numpy + installed concourse package.
"""

import numpy as np
from contextlib import ExitStack

import concourse.bass as bass
import concourse.bacc as bacc
import concourse.mybir as mybir
import concourse.tile as tile
from concourse.bass_utils import run_bass_kernel_spmd
from concourse.masks import make_identity, make_upper_triangular

# problem dims (hardcoded per harness contract)
B, L, D, C, NQ, KW, NL = 64, 1024, 256, 256, 10000, 4, 3
NCORES = 8
P = 128
F32 = mybir.dt.float32
BF16 = mybir.dt.bfloat16
F8 = mybir.dt.float8e4
I32 = mybir.dt.int32
AF = mybir.ActivationFunctionType
ALU = mybir.AluOpType
DR = mybir.MatmulPerfMode.DoubleRow

LT = L // P            # 8 token tiles of 128
NKT = (4 * D + C) // P  # 10 feature tiles of H^T
PAD = KW - 1           # 3 causal pad cols
XW = 1040              # padded x row (PAD+L rounded to 16) for DR pair stride
SC = 16.0              # value-domain scale for fp8 features/weights
MUTW = L
MMW = 4 * P


def _emit(nc, tc, ctx, dram, nb, repeat=1):
    sb = ctx.enter_context(tc.tile_pool(name="sb", bufs=1))
    seq = ctx.enter_context(tc.tile_pool(name="seq", bufs=1))
    wk = ctx.enter_context(tc.tile_pool(name="wk", bufs=1))
    ps = ctx.enter_context(tc.tile_pool(name="ps", bufs=1, space="PSUM"))

    # ---------------- constants ----------------
    ident8 = sb.tile([P, P], F8, tag="ident8")
    make_identity(nc, ident8[:])
    ident32 = sb.tile([P, P], F32, tag="ident32")
    make_identity(nc, ident32[:])
    smask8 = sb.tile([P, P], F8, tag="smask8")  # 1.0 where part < free (strict)
    make_upper_triangular(nc, smask8[:], val=1.0, diag=False)
    ones16 = sb.tile([P, 1], BF16, tag="ones16")
    nc.gpsimd.memset(ones16[:], 1.0)
    ones_row = sb.tile([1, P], BF16, tag="ones_row")
    nc.gpsimd.memset(ones_row[:], 1.0)
    ones8p = sb.tile([P, 2, 16], F8, tag="ones8p")  # DR ones: col 0 of each half
    nc.gpsimd.memset(ones8p[:], 0.0)
    nc.gpsimd.memset(ones8p[:, :, 0:1], 1.0)

    # biases (structurally zero in this problem, but honored):
    # MLP stt scalar = 256*w1b ; tanh bias = w2b/2
    # conv stt scalar = 16*convb_a ; tanh bias = convb_b/2
    w1b256 = sb.tile([P, 2], F32, tag="w1b256")
    w2bh = sb.tile([P, 2], F32, tag="w2bh")
    for dh in range(2):
        stg = wk.tile([P, 1], F32, tag="bstg", bufs=6, name=f"bstg1_{dh}")
        nc.sync.dma_start(out=stg[:], in_=dram["w1b"][dh * P:(dh + 1) * P, None])
        nc.vector.tensor_scalar_mul(w1b256[:, dh:dh + 1], stg[:], 256.0)
        stg2 = wk.tile([P, 1], F32, tag="bstg", bufs=6, name=f"bstg2_{dh}")
        nc.sync.dma_start(out=stg2[:], in_=dram["w2b"][dh * P:(dh + 1) * P, None])
        nc.vector.tensor_scalar_mul(w2bh[:, dh:dh + 1], stg2[:], 0.5)
    cba16 = sb.tile([P, NL * 2], F32, tag="cba16")   # 16*convb[a-half oc=pair]
    cbbh = sb.tile([P, NL * 2], F32, tag="cbbh")     # convb[b-half oc=2+pair]/2
    for ly in range(NL):
        for pair in range(2):
            stga = wk.tile([P, 1], F32, tag="bstg", bufs=6, name=f"cba{ly}_{pair}")
            nc.sync.dma_start(out=stga[:], in_=dram["convb"][ly, pair * P:(pair + 1) * P, None])
            nc.vector.tensor_scalar_mul(cba16[:, ly * 2 + pair:ly * 2 + pair + 1], stga[:], 16.0)
            stgb = wk.tile([P, 1], F32, tag="bstg", bufs=6, name=f"cbb{ly}_{pair}")
            nc.sync.dma_start(out=stgb[:], in_=dram["convb"][ly, (2 + pair) * P:(3 + pair) * P, None])
            nc.vector.tensor_scalar_mul(cbbh[:, ly * 2 + pair:ly * 2 + pair + 1], stgb[:], 0.5)

    # Ec columns: ec0T16 = 16*Ec[0,d], diffT16 = 16*(Ec[1,d]-Ec[0,d]) as [128,1] per dh
    ec0T16 = sb.tile([P, 2], F32, tag="ec0T16")
    diffT16 = sb.tile([P, 2], F32, tag="diffT16")
    for dh in range(2):
        e0 = wk.tile([P, 1], F32, tag="bstg", bufs=6, name=f"e0_{dh}")
        e1 = wk.tile([P, 1], F32, tag="bstg", bufs=6, name=f"e1_{dh}")
        nc.sync.dma_start(out=e0[:], in_=dram["ec"][0, dh * P:(dh + 1) * P, None])
        nc.sync.dma_start(out=e1[:], in_=dram["ec"][1, dh * P:(dh + 1) * P, None])
        nc.vector.tensor_scalar_mul(ec0T16[:, dh:dh + 1], e0[:], 16.0)
        d16 = wk.tile([P, 1], F32, tag="d16", bufs=2, name=f"d16_{dh}")
        nc.vector.tensor_sub(d16[:], e1[:], e0[:])
        nc.vector.tensor_scalar_mul(diffT16[:, dh:dh + 1], d16[:], 16.0)

    # ---------------- ce row constants (x16, broadcast across partitions) ----
    ec0_rep = sb.tile([P, D], BF16, tag="ec0_rep")
    diff_rep = sb.tile([P, D], BF16, tag="diff_rep")

    def emit_tables():
        e0r = wk.tile([1, D], F32, tag="ecrow", bufs=3, name="e0r")
        nc.sync.dma_start(out=e0r[:], in_=dram["ec"][0:1, :])
        e1r = wk.tile([1, D], F32, tag="ecrow", bufs=3, name="e1r")
        nc.sync.dma_start(out=e1r[:], in_=dram["ec"][1:2, :])
        dro = wk.tile([1, D], F32, tag="ecrow", bufs=3, name="drow")
        nc.vector.tensor_sub(dro[:], e1r[:], e0r[:])
        d16r = wk.tile([1, D], BF16, tag="ecrow16", bufs=2, name="d16r")
        nc.vector.tensor_scalar_mul(d16r[:], dro[:], SC)
        e16r = wk.tile([1, D], BF16, tag="ecrow16", bufs=2, name="e16r")
        nc.vector.tensor_scalar_mul(e16r[:], e0r[:], SC)
        nc.gpsimd.partition_broadcast(diff_rep[:], d16r[:])
        nc.gpsimd.partition_broadcast(ec0_rep[:], e16r[:])

    # ---------------- fp8 weights in SBUF ----------------
    # w1t8/w2t8: [k-feat 128, kt, dout 256] e4m3 x16  (kt pair stride 256)
    w1t8 = sb.tile([P, NKT, D], F8, tag="w1t8")
    w2t8 = sb.tile([P, NKT, D], F8, tag="w2t8")
    # cw8: [(ly,k), cin, 512] e4m3 x16  (cin pair stride 512)
    cw8 = sb.tile([P, NL * KW, 2, 2 * D], F8, tag="cw8")

    def emit_weights():
        for name, wt in (("w1w", w1t8), ("w2w", w2t8)):
            for dh in range(2):
                stg = wk.tile([P, NKT * P], F32, tag="wstage", bufs=2, name=f"wstg_{name}{dh}")
                nc.sync.dma_start(out=stg[:], in_=dram[name][dh * P:(dh + 1) * P, :])
                for kt in range(NKT):
                    tp = ps.tile([P, 4 * P], F32, tag="mma", bufs=4, name=f"wtp_{name}{dh}_{kt}")
                    nc.tensor.transpose(out=tp[:, 0:P], in_=stg[:, kt * P:(kt + 1) * P], identity=ident32[:])
                    nc.vector.tensor_scalar_mul(wt[:, kt, dh * P:(dh + 1) * P], tp[:, 0:P], SC)
        for ly in range(NL):
            for k in range(KW):
                for cin in range(2):
                    stg = wk.tile([P, 2 * D], F32, tag="cwstage", bufs=3, name=f"cwstg{ly}_{k}_{cin}")
                    nc.sync.dma_start(out=stg[:], in_=dram["convw"][ly, k, cin * P:(cin + 1) * P, :])
                    nc.vector.tensor_scalar_mul(cw8[:, ly * KW + k, cin, :], stg[:], SC)

    # ---------------- per-sequence pipeline stages ----------------
    issued = {}
    _dbg_tiles = {}

    def prep_issue(bg):
        """DMA-only: gathers + staging loads for sequence bg."""
        # qraw: gathered Eq rows in f32 [j 128, lt-major d]
        qraw = seq.tile([P, LT * D], F32, tag="qraw", bufs=2, name=f"qraw{bg}")
        # lis8: [j 128, jb, d 512] = [qe8 | ce8] x16 e4m3 (ce gathered direct)
        lis8 = seq.tile([P, LT, 2 * D], F8, tag="lis8", bufs=2, name=f"lis8_{bg}")
        ccols = wk.tile([P, LT], I32, tag="ccols", bufs=4, name=f"ccols{bg}")
        nc.sync.dma_start(
            out=ccols[:], in_=dram["cseq"][bg].rearrange("(lt p) -> p lt", p=P)
        )
        idxs = wk.tile([P, LT], I32, tag="idxs", bufs=4, name=f"idxs{bg}")
        nc.sync.dma_start(
            out=idxs[:], in_=dram["qseq"][bg].rearrange("(lt p) -> p lt", p=P)
        )
        for lt in range(LT):
            nc.gpsimd.indirect_dma_start(
                out=qraw[:, lt * D:(lt + 1) * D], out_offset=None, in_=dram["eq"][:],
                in_offset=bass.IndirectOffsetOnAxis(ap=idxs[:, lt:lt + 1], axis=0),
            )

        cqcs = []
        for ct in range(2):
            stg = wk.tile([P, L], F32, tag="cqcstage", bufs=4, name=f"cqcstg{bg}_{ct}")
            nc.sync.dma_start(out=stg[:], in_=dram["cqct"][bg, ct * P:(ct + 1) * P, :])
            cqcs.append(stg)
        corr_i = wk.tile([1, L], I32, tag="corr_i", bufs=4, name=f"corri{bg}")
        nc.sync.dma_start(out=corr_i[:], in_=dram["cseq"][bg:bg + 1, :])
        issued[bg] = (qraw, lis8, cqcs, corr_i, ccols)

    def prep_qe(bg):
        """Minimal pre-attention prep: lis8 qe-half + ht8 qeT blocks."""
        qraw, lis8, cqcs, corr_i, ccols = issued.pop(bg)
        # ht8 [feat 128, kt, l 1024]: kt 0-1 qeT, 2-3 ceT, 4-7 hrpT, 8-9 cqcT
        ht8 = seq.tile([P, NKT, L], F8, tag="ht8", bufs=2, name=f"ht8_{bg}")
        _dbg_tiles[bg] = {"ht8": ht8, "lis8": lis8, "qraw": qraw}

        # qe -> lis8 (x16, e4m3): one strided cast (flat traversal orders match)
        nc.gpsimd.tensor_scalar_mul(lis8[:, :, 0:D], qraw[:], SC)
        # ce -> lis8: ec0_rep + c*diff_rep (x16) per token block
        ccf = wk.tile([P, LT], F32, tag="ccf", bufs=2, name=f"ccf{bg}")
        nc.vector.tensor_copy(ccf[:], ccols[:])
        for lt in range(LT):
            nc.vector.scalar_tensor_tensor(
                out=lis8[:, lt, D:2 * D], in0=diff_rep[:], scalar=ccf[:, lt:lt + 1],
                in1=ec0_rep[:], op0=ALU.mult, op1=ALU.add,
            )

        for ltp in range(LT // 2):
            tp = ps.tile([P, 4 * P], F32, tag="mma", bufs=4, name=f"qtp{bg}_{ltp}")
            for dh in range(2):
                for i in range(2):
                    nc.tensor.transpose(
                        out=tp[:, dh * 2 * P + i * P:dh * 2 * P + (i + 1) * P],
                        in_=qraw[:, (2 * ltp + i) * D + dh * P:(2 * ltp + i) * D + (dh + 1) * P],
                        identity=ident32[:],
                    )
            # one strided scale-cast: psum [dh, ltpair 256] -> ht8[:, dh, ltp*256]
            nc.vector.tensor_scalar_mul(
                ht8[:, 0:2, 2 * ltp * P:(2 * ltp + 2) * P], tp[:], SC
            )
        return lis8, ht8, cqcs, corr_i

    def prep_rest(bg, ht8, cqcs, corr_i):
        """ceT + cqcT feature blocks (needed only by the MLP)."""
        for ct in range(2):
            nc.scalar.activation(ht8[:, 8 + ct, :], cqcs[ct][:], AF.Identity, scale=SC)

        corr16 = wk.tile([1, L], BF16, tag="corr16", bufs=4, name=f"corr16_{bg}")
        nc.vector.tensor_copy(corr16[:], corr_i[:])
        # broadcast c across partitions via PE; ceT acts read the psum
        crep = ps.tile([P, L], F32, tag="mmw", bufs=2, name=f"crep{bg}")
        for lt2 in range(2):
            nc.tensor.matmul(
                crep[:, lt2 * 4 * P:(lt2 + 1) * 4 * P], lhsT=ones_row[:],
                rhs=corr16[:, lt2 * 4 * P:(lt2 + 1) * 4 * P], start=True, stop=True,
            )
        for dh in range(2):
            nc.scalar.activation(
                ht8[:, 2 + dh, :], crep[:], AF.Identity,
                bias=ec0T16[:, dh:dh + 1], scale=diffT16[:, dh:dh + 1],
            )

    def attn(bg, lis8, ht8):
        """Scores (DR) -> exp -> T8 wide; hrpT directly transposed (DR)."""
        for iw in range(2):
            jmax = iw * 4 + 4
            i0 = iw * 4 * P
            tw = seq.tile([P, jmax, 4 * P], F8, tag=f"tw{iw}", bufs=2, name=f"tw{bg}_{iw}")
            _dbg_tiles[bg][f"tw{iw}"] = tw
            for jb in range(jmax):
                rel = max(jb * P - i0, 0)
                n_live = 4 * P - rel
                scp = ps.tile([P, 4 * P], F32, tag="mma", bufs=4, name=f"scp{bg}_{iw}_{jb}")
                nc.tensor.matmul(
                    scp[:, 0:n_live],
                    lhsT=ht8[:, 0:2, jb * P:(jb + 1) * P],
                    rhs=ht8[:, 0:2, i0 + rel:i0 + 4 * P],
                    start=True, stop=True, perf_mode=DR,
                )
                if rel > 0:
                    nc.gpsimd.memset(tw[:, jb, 0:rel], 0.0)
                nc.scalar.activation(tw[:, jb, rel:4 * P], scp[:, 0:n_live], AF.Exp,
                                     scale=1.0 / (SC * SC))
                if jb * P - i0 >= 0:
                    nc.vector.tensor_mul(tw[:, jb, rel:rel + P], tw[:, jb, rel:rel + P], smask8[:])
            # s = col-sums of T8 (DR ones)
            s_ps = ps.tile([1, 4 * P], F32, tag="mma", bufs=4, name=f"sps{bg}_{iw}")
            nhp = jmax // 2
            for jp in range(nhp):
                nc.tensor.matmul(
                    s_ps[:], lhsT=ones8p[:, :, 0:1], rhs=tw[:, 2 * jp:2 * jp + 2, :],
                    start=(jp == 0), stop=(jp == nhp - 1), perf_mode=DR,
                )
            sp = wk.tile([1, 4 * P], F32, tag="sp", bufs=2, name=f"sp{bg}_{iw}")
            nc.vector.tensor_scalar_add(sp[:], s_ps[:], 1e-8)
            nc.vector.reciprocal(sp[:], sp[:])
            srp = wk.tile([P, 4 * P], F32, tag="srp", bufs=2, name=f"srp{bg}_{iw}")
            nc.gpsimd.partition_broadcast(srp[:], sp[:])
            # hrpT: [d 128, i 512] per dh; psum = T8 x lis8(x16); norm by 1/s
            for dh in range(4):
                hp = ps.tile([P, 4 * P], F32, tag="mma", bufs=4, name=f"hp{bg}_{iw}_{dh}")
                for jp in range(nhp):
                    nc.tensor.matmul(
                        hp[:],
                        lhsT=lis8[:, 2 * jp:2 * jp + 2, dh * P:(dh + 1) * P],
                        rhs=tw[:, 2 * jp:2 * jp + 2, :],
                        start=(jp == 0), stop=(jp == nhp - 1), perf_mode=DR,
                    )
                nc.vector.tensor_mul(ht8[:, 4 + dh, i0:i0 + 4 * P], hp[:], srp[:])

    def mlp(bg, ht8):
        """Q = (H@W1+b1)*sigmoid(H@W2+b2) -> x8 (e4m3, x1 domain)."""
        x8 = seq.tile([P, 2, XW], F8, tag="xbuf", bufs=6, name=f"xq{bg}")
        _dbg_tiles[bg]["xq"] = x8
        for h in range(2):
            nc.gpsimd.memset(x8[:, h, 0:PAD], 0.0)
            nc.gpsimd.memset(x8[:, h, PAD + L:XW], 0.0)
        for dh in range(2):
            p2 = ps.tile([P, L], F32, tag="mmw", bufs=2, name=f"p2_{bg}_{dh}")
            for m in range(NKT // 2):
                for lt2 in range(2):
                    nc.tensor.matmul(
                        p2[:, lt2 * 4 * P:lt2 * 4 * P + MMW],
                        lhsT=w2t8[:, 2 * m:2 * m + 2, dh * P:(dh + 1) * P],
                        rhs=ht8[:, 2 * m:2 * m + 2, lt2 * 4 * P:lt2 * 4 * P + MMW],
                        start=(m == 0), stop=(m == NKT // 2 - 1), perf_mode=DR,
                    )
            gt = wk.tile([P, L], BF16, tag="gate", bufs=4, name=f"gt{bg}_{dh}")
            nc.scalar.activation(gt[:, 0:MUTW], p2[:, 0:MUTW], AF.Tanh,
                                 bias=w2bh[:, dh:dh + 1], scale=1.0 / 512.0)
            ge = wk.tile([P, L], BF16, tag="gate", bufs=4, name=f"ge{bg}_{dh}")
            nc.vector.tensor_scalar(ge[:, 0:MUTW], gt[:, 0:MUTW], 1.0 / 512.0, 1.0 / 512.0, ALU.mult, ALU.add)
            p1 = ps.tile([P, L], F32, tag="mmw", bufs=2, name=f"p1_{bg}_{dh}")
            for m in range(NKT // 2):
                for lt2 in range(2):
                    nc.tensor.matmul(
                        p1[:, lt2 * 4 * P:lt2 * 4 * P + MMW],
                        lhsT=w1t8[:, 2 * m:2 * m + 2, dh * P:(dh + 1) * P],
                        rhs=ht8[:, 2 * m:2 * m + 2, lt2 * 4 * P:lt2 * 4 * P + MMW],
                        start=(m == 0), stop=(m == NKT // 2 - 1), perf_mode=DR,
                    )
            nc.vector.scalar_tensor_tensor(
                out=x8[:, dh, PAD:PAD + MUTW], in0=p1[:, 0:MUTW], scalar=w1b256[:, dh:dh + 1],
                in1=ge[:, 0:MUTW], op0=ALU.add, op1=ALU.mult,
            )
        return x8

    def conv_layer(bg, ly, xcur):
        """One GLU-gated causal conv layer, residual in e4m3 x1."""
        if True:
            xnext = seq.tile([P, 2, XW], F8, tag="xbuf", bufs=6, name=f"xn{bg}_{ly}")
            for h in range(2):
                nc.gpsimd.memset(xnext[:, h, 0:PAD], 0.0)
                nc.gpsimd.memset(xnext[:, h, PAD + L:XW], 0.0)
            for pair in range(2):
                oc_a, oc_b = pair, 2 + pair
                pb = ps.tile([P, L], F32, tag="mmw", bufs=2, name=f"pb{bg}_{ly}_{pair}")
                for k in range(KW):
                    for lt2 in range(2):
                        nc.tensor.matmul(
                            pb[:, lt2 * 4 * P:lt2 * 4 * P + MMW],
                            lhsT=cw8[:, ly * KW + k, :, oc_b * P:(oc_b + 1) * P],
                            rhs=xcur[:, :, lt2 * 4 * P + k:lt2 * 4 * P + k + MMW],
                            start=(k == 0), stop=(k == KW - 1), perf_mode=DR,
                        )
                gt = wk.tile([P, L], BF16, tag="gate", bufs=4, name=f"cgt{bg}_{ly}_{pair}")
                nc.scalar.activation(gt[:, 0:MUTW], pb[:, 0:MUTW], AF.Tanh,
                                     bias=cbbh[:, ly * 2 + pair:ly * 2 + pair + 1],
                                     scale=1.0 / 32.0)
                ge = wk.tile([P, L], BF16, tag="gate", bufs=4, name=f"cge{bg}_{ly}_{pair}")
                nc.vector.tensor_scalar(ge[:, 0:MUTW], gt[:, 0:MUTW], 1.0 / 32.0, 1.0 / 32.0, ALU.mult, ALU.add)
                pa = ps.tile([P, L], F32, tag="mmw", bufs=2, name=f"pa{bg}_{ly}_{pair}")
                for k in range(KW):
                    for lt2 in range(2):
                        nc.tensor.matmul(
                            pa[:, lt2 * 4 * P:lt2 * 4 * P + MMW],
                            lhsT=cw8[:, ly * KW + k, :, oc_a * P:(oc_a + 1) * P],
                            rhs=xcur[:, :, lt2 * 4 * P + k:lt2 * 4 * P + k + MMW],
                            start=(k == 0), stop=(k == KW - 1), perf_mode=DR,
                        )
                glu = wk.tile([P, L], BF16, tag="glu", bufs=3, name=f"glu{bg}_{ly}_{pair}")
                nc.vector.scalar_tensor_tensor(
                    out=glu[:, 0:MUTW], in0=pa[:, 0:MUTW], scalar=cba16[:, ly * 2 + pair:ly * 2 + pair + 1],
                    in1=ge[:, 0:MUTW], op0=ALU.add, op1=ALU.mult,
                )
                nc.vector.tensor_add(
                    xnext[:, pair, PAD:PAD + MUTW], glu[:, 0:MUTW], xcur[:, pair, PAD:PAD + MUTW]
                )
        return xnext

    def predict(bg, xf, ht8):
        """predict = sigmoid(sum_d x[:-1] * qe[1:]) via tanh form."""
        prods = []
        for cin in range(2):
            prod = wk.tile([P, L - 1], BF16, tag="prod", bufs=2, name=f"prod{bg}_{cin}")
            nc.gpsimd.tensor_mul(
                prod[:], xf[:, cin, PAD:PAD + L - 1], ht8[:, cin, 1:L]
            )
            prods.append(prod)
        zp = ps.tile([1, L], F32, tag="mmw", bufs=2, name=f"zp{bg}")
        for half in range(2):
            n = 4 * P if half == 0 else L - 1 - 4 * P
            for cin in range(2):
                nc.tensor.matmul(
                    zp[:, half * 4 * P:half * 4 * P + n],
                    lhsT=ones16[:], rhs=prods[cin][:, half * 4 * P:half * 4 * P + n],
                    start=(cin == 0), stop=(cin == 1),
                )
        # zp = 16*z ; sigmoid(z) = 0.5*tanh(zp/32) + 0.5
        to = wk.tile([1, L - 1], F32, tag="osb", bufs=2, name=f"to{bg}")
        nc.scalar.activation(to[:], zp[:, 0:L - 1], AF.Tanh, scale=1.0 / 32.0)
        osb = wk.tile([1, L - 1], F32, tag="osb", bufs=2, name=f"osb{bg}")
        nc.vector.tensor_scalar(osb[:], to[:], 0.5, 0.5, ALU.mult, ALU.add)
        import os
        dbg = os.environ.get("KDBG")
        if dbg:
            src_ap = {
                "qet": lambda: _dbg_tiles[bg]["ht8"][0:1, 0, 0:L - 1],
                "cet": lambda: _dbg_tiles[bg]["ht8"][0:1, 2, 0:L - 1],
                "cqc": lambda: _dbg_tiles[bg]["ht8"][0:1, 8, 0:L - 1],
                "hrp": lambda: _dbg_tiles[bg]["ht8"][0:1, 4, 0:L - 1],
                "lisq": lambda: _dbg_tiles[bg]["lis8"][0:1, 0:2, 0:D],
                "lisc": lambda: _dbg_tiles[bg]["lis8"][0:1, 0:2, D:2 * D],
                "qraw": lambda: _dbg_tiles[bg]["qraw"][0:1, 0:L - 1],
                "tw1": lambda: _dbg_tiles[bg]["tw1"][0:1, :, :].rearrange("p a b -> p (a b)")[:, 0:L - 1],
                "xq": lambda: _dbg_tiles[bg]["xq"][0:1, 0, PAD:PAD + L - 1],
                "xf": lambda: xf[0:1, 0, PAD:PAD + L - 1],
            }[dbg]()
            dt = wk.tile([1, L - 1], F32, tag="osb", bufs=2, name=f"dbg{bg}")
            nfree = src_ap.free_size()
            nc.vector.memset(dt[:], 0.0)
            nc.vector.tensor_copy(dt[:, 0:nfree], src_ap)
            nc.sync.dma_start(out=dram["out"][bg:bg + 1, :], in_=dt[:])
        else:
            nc.sync.dma_start(out=dram["out"][bg:bg + 1, :], in_=osb[:])

    def pipeline():
        """Software-pipelined emission: the next sequence's prep/attention
        PE work is interleaved between the current sequence's conv layers so
        the in-order PE queue has fillable work during the conv's DVE/ACT
        dependency stalls."""
        prep_issue(0)
        st = {0: prep_qe(0)}
        attn(0, st[0][0], st[0][1])
        for bg in range(nb):
            lis8, ht8, cqcs, corr_i = st.pop(bg)
            prep_rest(bg, ht8, cqcs, corr_i)
            if bg + 1 < nb:
                prep_issue(bg + 1)
            x8 = mlp(bg, ht8)
            xcur = conv_layer(bg, 0, x8)
            if bg + 1 < nb:
                st[bg + 1] = prep_qe(bg + 1)
            xcur = conv_layer(bg, 1, xcur)
            if bg + 1 < nb:
                attn(bg + 1, st[bg + 1][0], st[bg + 1][1])
            xcur = conv_layer(bg, 2, xcur)
            predict(bg, xcur, ht8)

    # ---------------- emission schedule ----------------
    emit_tables()

    if repeat > 1:
        emit_weights()
        loop_cm = tc.For_i(0, repeat, 1)
        loop_cm.__enter__()
        pipeline()
        loop_cm.__exit__(None, None, None)
    else:
        emit_weights()
        pipeline()


# NOTE: inside the For_i repeat loop, prep_issue(0) is re-emitted each
# iteration (it is the first instruction group of the loop body), so the
# steady-state timing includes seq 0's DMA like every other sequence.


def build(nb, repeat=1):
    nc = bacc.Bacc("TRN2", target_bir_lowering=False, debug=False)
    dram = {
        "qseq": nc.dram_tensor("qseq", [nb, L], I32, kind="ExternalInput").ap(),
        "cseq": nc.dram_tensor("cseq", [nb, L], I32, kind="ExternalInput").ap(),
        "cqct": nc.dram_tensor("cqct", [nb, C, L], F32, kind="ExternalInput").ap(),
        "eq": nc.dram_tensor("eq", [NQ, D], F32, kind="ExternalInput").ap(),
        "ec": nc.dram_tensor("ec", [2, D], F32, kind="ExternalInput").ap(),
        "w1w": nc.dram_tensor("w1w", [D, 4 * D + C], F32, kind="ExternalInput").ap(),
        "w1b": nc.dram_tensor("w1b", [D], F32, kind="ExternalInput").ap(),
        "w2w": nc.dram_tensor("w2w", [D, 4 * D + C], F32, kind="ExternalInput").ap(),
        "w2b": nc.dram_tensor("w2b", [D], F32, kind="ExternalInput").ap(),
        "convw": nc.dram_tensor("convw", [NL, KW, D, 2 * D], F32, kind="ExternalInput").ap(),
        "convb": nc.dram_tensor("convb", [NL, 2 * D], F32, kind="ExternalInput").ap(),
        "out": nc.dram_tensor("out", [nb, L - 1], F32, kind="ExternalOutput").ap(),
    }
    with tile.TileContext(nc) as tc:
        with ExitStack() as ctx:
            _emit(nc, tc, ctx, dram, nb, repeat)
    nc.compile()
    return nc


_built = {}


def make_in_maps(inputs, nb):
    inp = {k: np.asarray(v) for k, v in inputs.items()}
    qseq = np.ascontiguousarray(inp["question_seq"].astype(np.int32))
    cseq = np.ascontiguousarray(inp["correctness_seq"].astype(np.int32))
    cqct = np.ascontiguousarray(
        np.transpose(inp["cqc_seq"].astype(np.float32), (0, 2, 1))
    )
    base = {
        "eq": np.ascontiguousarray(inp["Eq"].astype(np.float32)),
        "ec": np.ascontiguousarray(inp["Ec"].astype(np.float32)),
        "w1w": np.ascontiguousarray(inp["W1_w"].astype(np.float32)),
        "w1b": np.ascontiguousarray(inp["W1_b"].astype(np.float32)),
        "w2w": np.ascontiguousarray(inp["W2_w"].astype(np.float32)),
        "w2b": np.ascontiguousarray(inp["W2_b"].astype(np.float32)),
        "convw": np.ascontiguousarray(inp["conv_w"].astype(np.float32)),
        "convb": np.ascontiguousarray(inp["conv_b"].astype(np.float32)),
    }
    in_maps = []
    for cid in range(NCORES):
        sl = slice(cid * nb, (cid + 1) * nb)
        m = dict(base)
        m["qseq"] = qseq[sl]
        m["cseq"] = cseq[sl]
        m["cqct"] = cqct[sl]
        in_maps.append(m)
    return in_maps


def run_sharded(inputs, nb=B // NCORES, trace=False, **kw):
    if nb not in _built:
        _built[nb] = build(nb)
    nc = _built[nb]
    in_maps = make_in_maps(inputs, nb)
    res = run_bass_kernel_spmd(nc, in_maps, list(range(NCORES)), trace=trace, **kw)
    out = np.concatenate([res.results[c]["out"] for c in range(NCORES)], axis=0)
    return out.astype(np.float32), res


def kernel(**inputs):
    out, _ = run_sharded(inputs)
    return out


# revision 52
# speedup vs baseline: 1.3418x; 1.3418x over previous
"""Trainium2 Bass kernel for the dense_transformer problem.

Data-parallel over batch: 8 NeuronCores x (B/8) sequences each.

v2: fp8 (e4m3) DoubleRow matmuls everywhere big, x16 value domain for
features/weights (scale factors fold into activation scale slots and the
tanh-affine constants), tanh-form sigmoids so the whole kernel uses one
activation table set (exp_and_others), ce/ceT built without PE outer
products, hrpT computed directly in transposed orientation (no PE
transposes for HRP).

Self-contained: only imports numpy + installed concourse package.
"""

import numpy as np
from contextlib import ExitStack

import concourse.bass as bass
import concourse.bacc as bacc
import concourse.mybir as mybir
import concourse.tile as tile
from concourse.bass_utils import run_bass_kernel_spmd
from concourse.masks import make_identity, make_upper_triangular

# problem dims (hardcoded per harness contract)
B, L, D, C, NQ, KW, NL = 64, 1024, 256, 256, 10000, 4, 3
NCORES = 8
P = 128
F32 = mybir.dt.float32
BF16 = mybir.dt.bfloat16
F8 = mybir.dt.float8e4
I32 = mybir.dt.int32
AF = mybir.ActivationFunctionType
ALU = mybir.AluOpType
DR = mybir.MatmulPerfMode.DoubleRow

LT = L // P            # 8 token tiles of 128
NKT = (4 * D + C) // P  # 10 feature tiles of H^T
PAD = KW - 1           # 3 causal pad cols
XW = 1040              # padded x row (PAD+L rounded to 16) for DR pair stride
SC = 16.0              # value-domain scale for fp8 features/weights
MUTW = L
MMW = 4 * P


def _emit(nc, tc, ctx, dram, nb, repeat=1):
    sb = ctx.enter_context(tc.tile_pool(name="sb", bufs=1))
    seq = ctx.enter_context(tc.tile_pool(name="seq", bufs=1))
    wk = ctx.enter_context(tc.tile_pool(name="wk", bufs=1))
    ps = ctx.enter_context(tc.tile_pool(name="ps", bufs=1, space="PSUM"))

    # ---------------- constants ----------------
    ident8 = sb.tile([P, P], F8, tag="ident8")
    make_identity(nc, ident8[:])
    ident32 = sb.tile([P, P], F32, tag="ident32")
    make_identity(nc, ident32[:])
    ident16b = sb.tile([P, P], BF16, tag="ident16b")
    make_identity(nc, ident16b[:])
    smask8 = sb.tile([P, P], F8, tag="smask8")  # 1.0 where part < free (strict)
    make_upper_triangular(nc, smask8[:], val=1.0, diag=False)
    ones16 = sb.tile([P, 1], BF16, tag="ones16")
    nc.gpsimd.memset(ones16[:], 1.0)
    ones_row = sb.tile([1, P], BF16, tag="ones_row")
    nc.gpsimd.memset(ones_row[:], 1.0)
    ones8p = sb.tile([P, 2, 16], F8, tag="ones8p")  # DR ones: col 0 of each half
    nc.gpsimd.memset(ones8p[:], 0.0)
    nc.gpsimd.memset(ones8p[:, :, 0:1], 1.0)

    # biases (structurally zero in this problem, but honored):
    # MLP stt scalar = 256*w1b ; tanh bias = w2b/2
    # conv stt scalar = 16*convb_a ; tanh bias = convb_b/2
    w1b256 = sb.tile([P, 2], F32, tag="w1b256")
    w2bh = sb.tile([P, 2], F32, tag="w2bh")
    for dh in range(2):
        stg = wk.tile([P, 1], F32, tag="bstg", bufs=6, name=f"bstg1_{dh}")
        nc.sync.dma_start(out=stg[:], in_=dram["w1b"][dh * P:(dh + 1) * P, None])
        nc.vector.tensor_scalar_mul(w1b256[:, dh:dh + 1], stg[:], 256.0)
        stg2 = wk.tile([P, 1], F32, tag="bstg", bufs=6, name=f"bstg2_{dh}")
        nc.sync.dma_start(out=stg2[:], in_=dram["w2b"][dh * P:(dh + 1) * P, None])
        nc.vector.tensor_scalar_mul(w2bh[:, dh:dh + 1], stg2[:], 0.5)
    cba16 = sb.tile([P, NL * 2], F32, tag="cba16")   # 16*convb[a-half oc=pair]
    cbbh = sb.tile([P, NL * 2], F32, tag="cbbh")     # convb[b-half oc=2+pair]/2
    for ly in range(NL):
        for pair in range(2):
            stga = wk.tile([P, 1], F32, tag="bstg", bufs=6, name=f"cba{ly}_{pair}")
            nc.sync.dma_start(out=stga[:], in_=dram["convb"][ly, pair * P:(pair + 1) * P, None])
            nc.vector.tensor_scalar_mul(cba16[:, ly * 2 + pair:ly * 2 + pair + 1], stga[:], 16.0)
            stgb = wk.tile([P, 1], F32, tag="bstg", bufs=6, name=f"cbb{ly}_{pair}")
            nc.sync.dma_start(out=stgb[:], in_=dram["convb"][ly, (2 + pair) * P:(3 + pair) * P, None])
            nc.vector.tensor_scalar_mul(cbbh[:, ly * 2 + pair:ly * 2 + pair + 1], stgb[:], 0.5)

    # Ec columns: ec0T16 = 16*Ec[0,d], diffT16 = 16*(Ec[1,d]-Ec[0,d]) as [128,1] per dh
    ec0T16 = sb.tile([P, 2], F32, tag="ec0T16")
    diffT16 = sb.tile([P, 2], F32, tag="diffT16")
    for dh in range(2):
        e0 = wk.tile([P, 1], F32, tag="bstg", bufs=6, name=f"e0_{dh}")
        e1 = wk.tile([P, 1], F32, tag="bstg", bufs=6, name=f"e1_{dh}")
        nc.sync.dma_start(out=e0[:], in_=dram["ec"][0, dh * P:(dh + 1) * P, None])
        nc.sync.dma_start(out=e1[:], in_=dram["ec"][1, dh * P:(dh + 1) * P, None])
        nc.vector.tensor_scalar_mul(ec0T16[:, dh:dh + 1], e0[:], 16.0)
        d16 = wk.tile([P, 1], F32, tag="d16", bufs=2, name=f"d16_{dh}")
        nc.vector.tensor_sub(d16[:], e1[:], e0[:])
        nc.vector.tensor_scalar_mul(diffT16[:, dh:dh + 1], d16[:], 16.0)

    # ---------------- ce row constants (x16, broadcast across partitions) ----
    ec0_rep = sb.tile([P, D], BF16, tag="ec0_rep")
    diff_rep = sb.tile([P, D], BF16, tag="diff_rep")

    def emit_tables():
        e0r = wk.tile([1, D], F32, tag="ecrow", bufs=3, name="e0r")
        nc.sync.dma_start(out=e0r[:], in_=dram["ec"][0:1, :])
        e1r = wk.tile([1, D], F32, tag="ecrow", bufs=3, name="e1r")
        nc.sync.dma_start(out=e1r[:], in_=dram["ec"][1:2, :])
        dro = wk.tile([1, D], F32, tag="ecrow", bufs=3, name="drow")
        nc.vector.tensor_sub(dro[:], e1r[:], e0r[:])
        d16r = wk.tile([1, D], BF16, tag="ecrow16", bufs=2, name="d16r")
        nc.vector.tensor_scalar_mul(d16r[:], dro[:], SC)
        e16r = wk.tile([1, D], BF16, tag="ecrow16", bufs=2, name="e16r")
        nc.vector.tensor_scalar_mul(e16r[:], e0r[:], SC)
        nc.gpsimd.partition_broadcast(diff_rep[:], d16r[:])
        nc.gpsimd.partition_broadcast(ec0_rep[:], e16r[:])

    # ---------------- fp8 weights in SBUF ----------------
    # w1t8/w2t8: [k-feat 128, kt, dout 256] e4m3 x16  (kt pair stride 256)
    w1t8 = sb.tile([P, NKT, D], F8, tag="w1t8")
    w2t8 = sb.tile([P, NKT, D], F8, tag="w2t8")
    # cw8: [(ly,k), cin, 512] e4m3 x16  (cin pair stride 512)
    cw8 = sb.tile([P, NL * KW, 2, 2 * D], F8, tag="cw8")

    def emit_weights():
        for name, wt in (("w1w", w1t8), ("w2w", w2t8)):
            for dh in range(2):
                stg = wk.tile([P, NKT * P], F32, tag="wstage", bufs=2, name=f"wstg_{name}{dh}")
                nc.sync.dma_start(out=stg[:], in_=dram[name][dh * P:(dh + 1) * P, :])
                for kt in range(NKT):
                    tp = ps.tile([P, 4 * P], F32, tag="mma", bufs=4, name=f"wtp_{name}{dh}_{kt}")
                    nc.tensor.transpose(out=tp[:, 0:P], in_=stg[:, kt * P:(kt + 1) * P], identity=ident32[:])
                    nc.vector.tensor_scalar_mul(wt[:, kt, dh * P:(dh + 1) * P], tp[:, 0:P], SC)
        for ly in range(NL):
            for k in range(KW):
                for cin in range(2):
                    stg = wk.tile([P, 2 * D], F32, tag="cwstage", bufs=3, name=f"cwstg{ly}_{k}_{cin}")
                    nc.sync.dma_start(out=stg[:], in_=dram["convw"][ly, k, cin * P:(cin + 1) * P, :])
                    nc.vector.tensor_scalar_mul(cw8[:, ly * KW + k, cin, :], stg[:], SC)

    # ---------------- per-sequence pipeline stages ----------------
    issued = {}
    _dbg_tiles = {}

    def prep_issue(bg):
        """DMA-only: gathers + staging loads for sequence bg."""
        # qraw: gathered Eq rows in f32 [j 128, lt-major d]
        qraw = seq.tile([P, LT * D], F32, tag="qraw", bufs=2, name=f"qraw{bg}")
        # lis8: [j 128, jb, d 512] = [qe8 | ce8] x16 e4m3 (ce gathered direct)
        lis8 = seq.tile([P, LT, 2 * D], F8, tag="lis8", bufs=2, name=f"lis8_{bg}")
        ccols = wk.tile([P, LT], I32, tag="ccols", bufs=4, name=f"ccols{bg}")
        nc.sync.dma_start(
            out=ccols[:], in_=dram["cseq"][bg].rearrange("(lt p) -> p lt", p=P)
        )
        idxs = wk.tile([P, LT], I32, tag="idxs", bufs=4, name=f"idxs{bg}")
        nc.sync.dma_start(
            out=idxs[:], in_=dram["qseq"][bg].rearrange("(lt p) -> p lt", p=P)
        )
        for lt in range(LT):
            nc.gpsimd.indirect_dma_start(
                out=qraw[:, lt * D:(lt + 1) * D], out_offset=None, in_=dram["eq"][:],
                in_offset=bass.IndirectOffsetOnAxis(ap=idxs[:, lt:lt + 1], axis=0),
            )

        cqcs = []
        for ct in range(2):
            stg = wk.tile([P, L], F32, tag="cqcstage", bufs=4, name=f"cqcstg{bg}_{ct}")
            nc.sync.dma_start(out=stg[:], in_=dram["cqct"][bg, ct * P:(ct + 1) * P, :])
            cqcs.append(stg)
        corr_i = wk.tile([1, L], I32, tag="corr_i", bufs=4, name=f"corri{bg}")
        nc.sync.dma_start(out=corr_i[:], in_=dram["cseq"][bg:bg + 1, :])
        issued[bg] = (qraw, lis8, cqcs, corr_i, ccols)

    def prep_qe(bg):
        """Minimal pre-attention prep: lis8 qe-half + ht8 qeT blocks."""
        qraw, lis8, cqcs, corr_i, ccols = issued.pop(bg)
        # ht8 [feat 128, kt, l 1024]: kt 0-1 qeT, 2-3 ceT, 4-7 hrpT, 8-9 cqcT
        ht8 = seq.tile([P, NKT, L], F8, tag="ht8", bufs=2, name=f"ht8_{bg}")
        _dbg_tiles[bg] = {"ht8": ht8, "lis8": lis8, "qraw": qraw}

        # qe -> lis8 (x16, e4m3): one strided cast (flat traversal orders match)
        nc.gpsimd.tensor_scalar_mul(lis8[:, :, 0:D], qraw[:], SC)
        # qe16: bf16 x16 staging so the qeT transposes run at bf16 rate
        qe16 = wk.tile([P, LT * D], BF16, tag="qe16", bufs=2, name=f"qe16_{bg}")
        nc.scalar.activation(qe16[:], qraw[:], AF.Identity, scale=SC)
        # ce -> lis8: ec0_rep + c*diff_rep (x16) per token block
        ccf = wk.tile([P, LT], F32, tag="ccf", bufs=2, name=f"ccf{bg}")
        nc.vector.tensor_copy(ccf[:], ccols[:])
        for lt in range(LT):
            nc.vector.scalar_tensor_tensor(
                out=lis8[:, lt, D:2 * D], in0=diff_rep[:], scalar=ccf[:, lt:lt + 1],
                in1=ec0_rep[:], op0=ALU.mult, op1=ALU.add,
            )

        for ltp in range(LT // 2):
            tp = ps.tile([P, 4 * P], BF16, tag="mma", bufs=4, name=f"qtp{bg}_{ltp}")
            for dh in range(2):
                for i in range(2):
                    nc.tensor.transpose(
                        out=tp[:, dh * 2 * P + i * P:dh * 2 * P + (i + 1) * P],
                        in_=qe16[:, (2 * ltp + i) * D + dh * P:(2 * ltp + i) * D + (dh + 1) * P],
                        identity=ident16b[:],
                    )
            # one strided cast: psum bf16 (already x16) -> ht8 e4m3
            nc.vector.tensor_copy(
                ht8[:, 0:2, 2 * ltp * P:(2 * ltp + 2) * P], tp[:]
            )
        return lis8, ht8, cqcs, corr_i

    def prep_rest(bg, ht8, cqcs, corr_i):
        """ceT + cqcT feature blocks (needed only by the MLP)."""
        for ct in range(2):
            nc.scalar.activation(ht8[:, 8 + ct, :], cqcs[ct][:], AF.Identity, scale=SC)

        corr16 = wk.tile([1, L], BF16, tag="corr16", bufs=4, name=f"corr16_{bg}")
        nc.vector.tensor_copy(corr16[:], corr_i[:])
        # broadcast c across partitions via PE; ceT acts read the psum
        crep = ps.tile([P, L], F32, tag="mmw", bufs=2, name=f"crep{bg}")
        for lt2 in range(2):
            nc.tensor.matmul(
                crep[:, lt2 * 4 * P:(lt2 + 1) * 4 * P], lhsT=ones_row[:],
                rhs=corr16[:, lt2 * 4 * P:(lt2 + 1) * 4 * P], start=True, stop=True,
            )
        for dh in range(2):
            nc.scalar.activation(
                ht8[:, 2 + dh, :], crep[:], AF.Identity,
                bias=ec0T16[:, dh:dh + 1], scale=diffT16[:, dh:dh + 1],
            )

    def attn(bg, lis8, ht8):
        """Scores (DR) -> exp -> T8 wide; hrpT directly transposed (DR)."""
        for iw in range(2):
            jmax = iw * 4 + 4
            i0 = iw * 4 * P
            tw = seq.tile([P, jmax, 4 * P], F8, tag=f"tw{iw}", bufs=2, name=f"tw{bg}_{iw}")
            _dbg_tiles[bg][f"tw{iw}"] = tw
            for jb in range(jmax):
                rel = max(jb * P - i0, 0)
                n_live = 4 * P - rel
                scp = ps.tile([P, 4 * P], F32, tag="mma", bufs=4, name=f"scp{bg}_{iw}_{jb}")
                nc.tensor.matmul(
                    scp[:, 0:n_live],
                    lhsT=ht8[:, 0:2, jb * P:(jb + 1) * P],
                    rhs=ht8[:, 0:2, i0 + rel:i0 + 4 * P],
                    start=True, stop=True, perf_mode=DR,
                )
                if rel > 0:
                    nc.gpsimd.memset(tw[:, jb, 0:rel], 0.0)
                nc.scalar.activation(tw[:, jb, rel:4 * P], scp[:, 0:n_live], AF.Exp,
                                     scale=1.0 / (SC * SC))
                if jb * P - i0 >= 0:
                    nc.vector.tensor_mul(tw[:, jb, rel:rel + P], tw[:, jb, rel:rel + P], smask8[:])
            # s = col-sums of T8 (DR ones)
            s_ps = ps.tile([1, 4 * P], F32, tag="mma", bufs=4, name=f"sps{bg}_{iw}")
            nhp = jmax // 2
            for jp in range(nhp):
                nc.tensor.matmul(
                    s_ps[:], lhsT=ones8p[:, :, 0:1], rhs=tw[:, 2 * jp:2 * jp + 2, :],
                    start=(jp == 0), stop=(jp == nhp - 1), perf_mode=DR,
                )
            sp = wk.tile([1, 4 * P], F32, tag="sp", bufs=2, name=f"sp{bg}_{iw}")
            nc.vector.tensor_scalar_add(sp[:], s_ps[:], 1e-8)
            nc.vector.reciprocal(sp[:], sp[:])
            srp = wk.tile([P, 4 * P], F32, tag="srp", bufs=2, name=f"srp{bg}_{iw}")
            nc.gpsimd.partition_broadcast(srp[:], sp[:])
            # hrpT: [d 128, i 512] per dh; psum = T8 x lis8(x16); norm by 1/s
            for dh in range(4):
                hp = ps.tile([P, 4 * P], F32, tag="mma", bufs=4, name=f"hp{bg}_{iw}_{dh}")
                for jp in range(nhp):
                    nc.tensor.matmul(
                        hp[:],
                        lhsT=lis8[:, 2 * jp:2 * jp + 2, dh * P:(dh + 1) * P],
                        rhs=tw[:, 2 * jp:2 * jp + 2, :],
                        start=(jp == 0), stop=(jp == nhp - 1), perf_mode=DR,
                    )
                nc.vector.tensor_mul(ht8[:, 4 + dh, i0:i0 + 4 * P], hp[:], srp[:])

    def mlp(bg, ht8):
        """Q = (H@W1+b1)*sigmoid(H@W2+b2) -> x8 (e4m3, x1 domain)."""
        x8 = seq.tile([P, 2, XW], F8, tag="xbuf", bufs=6, name=f"xq{bg}")
        _dbg_tiles[bg]["xq"] = x8
        for h in range(2):
            nc.gpsimd.memset(x8[:, h, 0:PAD], 0.0)
            nc.gpsimd.memset(x8[:, h, PAD + L:XW], 0.0)
        for dh in range(2):
            p2 = ps.tile([P, L], F32, tag="mmw", bufs=2, name=f"p2_{bg}_{dh}")
            for m in range(NKT // 2):
                for lt2 in range(2):
                    nc.tensor.matmul(
                        p2[:, lt2 * 4 * P:lt2 * 4 * P + MMW],
                        lhsT=w2t8[:, 2 * m:2 * m + 2, dh * P:(dh + 1) * P],
                        rhs=ht8[:, 2 * m:2 * m + 2, lt2 * 4 * P:lt2 * 4 * P + MMW],
                        start=(m == 0), stop=(m == NKT // 2 - 1), perf_mode=DR,
                    )
            gt = wk.tile([P, L], BF16, tag="gate", bufs=4, name=f"gt{bg}_{dh}")
            nc.scalar.activation(gt[:, 0:MUTW], p2[:, 0:MUTW], AF.Tanh,
                                 bias=w2bh[:, dh:dh + 1], scale=1.0 / 512.0)
            ge = wk.tile([P, L], BF16, tag="gate", bufs=4, name=f"ge{bg}_{dh}")
            nc.vector.tensor_scalar(ge[:, 0:MUTW], gt[:, 0:MUTW], 1.0 / 512.0, 1.0 / 512.0, ALU.mult, ALU.add)
            p1 = ps.tile([P, L], F32, tag="mmw", bufs=2, name=f"p1_{bg}_{dh}")
            for m in range(NKT // 2):
                for lt2 in range(2):
                    nc.tensor.matmul(
                        p1[:, lt2 * 4 * P:lt2 * 4 * P + MMW],
                        lhsT=w1t8[:, 2 * m:2 * m + 2, dh * P:(dh + 1) * P],
                        rhs=ht8[:, 2 * m:2 * m + 2, lt2 * 4 * P:lt2 * 4 * P + MMW],
                        start=(m == 0), stop=(m == NKT // 2 - 1), perf_mode=DR,
                    )
            nc.vector.scalar_tensor_tensor(
                out=x8[:, dh, PAD:PAD + MUTW], in0=p1[:, 0:MUTW], scalar=w1b256[:, dh:dh + 1],
                in1=ge[:, 0:MUTW], op0=ALU.add, op1=ALU.mult,
            )
        return x8

    def conv_layer(bg, ly, xcur):
        """One GLU-gated causal conv layer, residual in e4m3 x1."""
        if True:
            xnext = seq.tile([P, 2, XW], F8, tag="xbuf", bufs=6, name=f"xn{bg}_{ly}")
            for h in range(2):
                nc.gpsimd.memset(xnext[:, h, 0:PAD], 0.0)
                nc.gpsimd.memset(xnext[:, h, PAD + L:XW], 0.0)
            for pair in range(2):
                oc_a, oc_b = pair, 2 + pair
                pb = ps.tile([P, L], F32, tag="mmw", bufs=2, name=f"pb{bg}_{ly}_{pair}")
                for k in range(KW):
                    for lt2 in range(2):
                        nc.tensor.matmul(
                            pb[:, lt2 * 4 * P:lt2 * 4 * P + MMW],
                            lhsT=cw8[:, ly * KW + k, :, oc_b * P:(oc_b + 1) * P],
                            rhs=xcur[:, :, lt2 * 4 * P + k:lt2 * 4 * P + k + MMW],
                            start=(k == 0), stop=(k == KW - 1), perf_mode=DR,
                        )
                gt = wk.tile([P, L], BF16, tag="gate", bufs=4, name=f"cgt{bg}_{ly}_{pair}")
                nc.scalar.activation(gt[:, 0:MUTW], pb[:, 0:MUTW], AF.Tanh,
                                     bias=cbbh[:, ly * 2 + pair:ly * 2 + pair + 1],
                                     scale=1.0 / 32.0)
                ge = wk.tile([P, L], BF16, tag="gate", bufs=4, name=f"cge{bg}_{ly}_{pair}")
                nc.vector.tensor_scalar(ge[:, 0:MUTW], gt[:, 0:MUTW], 1.0 / 32.0, 1.0 / 32.0, ALU.mult, ALU.add)
                pa = ps.tile([P, L], F32, tag="mmw", bufs=2, name=f"pa{bg}_{ly}_{pair}")
                for k in range(KW):
                    for lt2 in range(2):
                        nc.tensor.matmul(
                            pa[:, lt2 * 4 * P:lt2 * 4 * P + MMW],
                            lhsT=cw8[:, ly * KW + k, :, oc_a * P:(oc_a + 1) * P],
                            rhs=xcur[:, :, lt2 * 4 * P + k:lt2 * 4 * P + k + MMW],
                            start=(k == 0), stop=(k == KW - 1), perf_mode=DR,
                        )
                glu = wk.tile([P, L], BF16, tag="glu", bufs=3, name=f"glu{bg}_{ly}_{pair}")
                nc.vector.scalar_tensor_tensor(
                    out=glu[:, 0:MUTW], in0=pa[:, 0:MUTW], scalar=cba16[:, ly * 2 + pair:ly * 2 + pair + 1],
                    in1=ge[:, 0:MUTW], op0=ALU.add, op1=ALU.mult,
                )
                nc.vector.tensor_add(
                    xnext[:, pair, PAD:PAD + MUTW], glu[:, 0:MUTW], xcur[:, pair, PAD:PAD + MUTW]
                )
        return xnext

    def predict(bg, xf, ht8):
        """predict = sigmoid(sum_d x[:-1] * qe[1:]) via tanh form."""
        prods = []
        for cin in range(2):
            prod = wk.tile([P, L - 1], BF16, tag="prod", bufs=2, name=f"prod{bg}_{cin}")
            nc.gpsimd.tensor_mul(
                prod[:], xf[:, cin, PAD:PAD + L - 1], ht8[:, cin, 1:L]
            )
            prods.append(prod)
        zp = ps.tile([1, L], F32, tag="mmw", bufs=2, name=f"zp{bg}")
        for half in range(2):
            n = 4 * P if half == 0 else L - 1 - 4 * P
            for cin in range(2):
                nc.tensor.matmul(
                    zp[:, half * 4 * P:half * 4 * P + n],
                    lhsT=ones16[:], rhs=prods[cin][:, half * 4 * P:half * 4 * P + n],
                    start=(cin == 0), stop=(cin == 1),
                )
        # zp = 16*z ; sigmoid(z) = 0.5*tanh(zp/32) + 0.5
        to = wk.tile([1, L - 1], F32, tag="osb", bufs=2, name=f"to{bg}")
        nc.scalar.activation(to[:], zp[:, 0:L - 1], AF.Tanh, scale=1.0 / 32.0)
        osb = wk.tile([1, L - 1], F32, tag="osb", bufs=2, name=f"osb{bg}")
        nc.vector.tensor_scalar(osb[:], to[:], 0.5, 0.5, ALU.mult, ALU.add)
        import os
        dbg = os.environ.get("KDBG")
        if dbg:
            src_ap = {
                "qet": lambda: _dbg_tiles[bg]["ht8"][0:1, 0, 0:L - 1],
                "cet": lambda: _dbg_tiles[bg]["ht8"][0:1, 2, 0:L - 1],
                "cqc": lambda: _dbg_tiles[bg]["ht8"][0:1, 8, 0:L - 1],
                "hrp": lambda: _dbg_tiles[bg]["ht8"][0:1, 4, 0:L - 1],
                "lisq": lambda: _dbg_tiles[bg]["lis8"][0:1, 0:2, 0:D],
                "lisc": lambda: _dbg_tiles[bg]["lis8"][0:1, 0:2, D:2 * D],
                "qraw": lambda: _dbg_tiles[bg]["qraw"][0:1, 0:L - 1],
                "tw1": lambda: _dbg_tiles[bg]["tw1"][0:1, :, :].rearrange("p a b -> p (a b)")[:, 0:L - 1],
                "xq": lambda: _dbg_tiles[bg]["xq"][0:1, 0, PAD:PAD + L - 1],
                "xf": lambda: xf[0:1, 0, PAD:PAD + L - 1],
            }[dbg]()
            dt = wk.tile([1, L - 1], F32, tag="osb", bufs=2, name=f"dbg{bg}")
            nfree = src_ap.free_size()
            nc.vector.memset(dt[:], 0.0)
            nc.vector.tensor_copy(dt[:, 0:nfree], src_ap)
            nc.sync.dma_start(out=dram["out"][bg:bg + 1, :], in_=dt[:])
        else:
            nc.sync.dma_start(out=dram["out"][bg:bg + 1, :], in_=osb[:])

    def pipeline():
        """Software-pipelined emission: the next sequence's prep/attention
        PE work is interleaved between the current sequence's conv layers so
        the in-order PE queue has fillable work during the conv's DVE/ACT
        dependency stalls."""
        prep_issue(0)
        st = {0: prep_qe(0)}
        attn(0, st[0][0], st[0][1])
        for bg in range(nb):
            lis8, ht8, cqcs, corr_i = st.pop(bg)
            prep_rest(bg, ht8, cqcs, corr_i)
            if bg + 1 < nb:
                prep_issue(bg + 1)
            x8 = mlp(bg, ht8)
            xcur = conv_layer(bg, 0, x8)
            if bg + 1 < nb:
                st[bg + 1] = prep_qe(bg + 1)
            xcur = conv_layer(bg, 1, xcur)
            if bg + 1 < nb:
                attn(bg + 1, st[bg + 1][0], st[bg + 1][1])
            xcur = conv_layer(bg, 2, xcur)
            predict(bg, xcur, ht8)

    # ---------------- emission schedule ----------------
    emit_tables()

    if repeat > 1:
        emit_weights()
        loop_cm = tc.For_i(0, repeat, 1)
        loop_cm.__enter__()
        pipeline()
        loop_cm.__exit__(None, None, None)
    else:
        emit_weights()
        pipeline()


# NOTE: inside the For_i repeat loop, prep_issue(0) is re-emitted each
# iteration (it is the first instruction group of the loop body), so the
# steady-state timing includes seq 0's DMA like every other sequence.


def build(nb, repeat=1):
    nc = bacc.Bacc("TRN2", target_bir_lowering=False, debug=False)
    dram = {
        "qseq": nc.dram_tensor("qseq", [nb, L], I32, kind="ExternalInput").ap(),
        "cseq": nc.dram_tensor("cseq", [nb, L], I32, kind="ExternalInput").ap(),
        "cqct": nc.dram_tensor("cqct", [nb, C, L], F32, kind="ExternalInput").ap(),
        "eq": nc.dram_tensor("eq", [NQ, D], F32, kind="ExternalInput").ap(),
        "ec": nc.dram_tensor("ec", [2, D], F32, kind="ExternalInput").ap(),
        "w1w": nc.dram_tensor("w1w", [D, 4 * D + C], F32, kind="ExternalInput").ap(),
        "w1b": nc.dram_tensor("w1b", [D], F32, kind="ExternalInput").ap(),
        "w2w": nc.dram_tensor("w2w", [D, 4 * D + C], F32, kind="ExternalInput").ap(),
        "w2b": nc.dram_tensor("w2b", [D], F32, kind="ExternalInput").ap(),
        "convw": nc.dram_tensor("convw", [NL, KW, D, 2 * D], F32, kind="ExternalInput").ap(),
        "convb": nc.dram_tensor("convb", [NL, 2 * D], F32, kind="ExternalInput").ap(),
        "out": nc.dram_tensor("out", [nb, L - 1], F32, kind="ExternalOutput").ap(),
    }
    with tile.TileContext(nc) as tc:
        with ExitStack() as ctx:
            _emit(nc, tc, ctx, dram, nb, repeat)
    nc.compile()
    return nc


_built = {}


def make_in_maps(inputs, nb):
    inp = {k: np.asarray(v) for k, v in inputs.items()}
    qseq = np.ascontiguousarray(inp["question_seq"].astype(np.int32))
    cseq = np.ascontiguousarray(inp["correctness_seq"].astype(np.int32))
    cqct = np.ascontiguousarray(
        np.transpose(inp["cqc_seq"].astype(np.float32), (0, 2, 1))
    )
    base = {
        "eq": np.ascontiguousarray(inp["Eq"].astype(np.float32)),
        "ec": np.ascontiguousarray(inp["Ec"].astype(np.float32)),
        "w1w": np.ascontiguousarray(inp["W1_w"].astype(np.float32)),
        "w1b": np.ascontiguousarray(inp["W1_b"].astype(np.float32)),
        "w2w": np.ascontiguousarray(inp["W2_w"].astype(np.float32)),
        "w2b": np.ascontiguousarray(inp["W2_b"].astype(np.float32)),
        "convw": np.ascontiguousarray(inp["conv_w"].astype(np.float32)),
        "convb": np.ascontiguousarray(inp["conv_b"].astype(np.float32)),
    }
    in_maps = []
    for cid in range(NCORES):
        sl = slice(cid * nb, (cid + 1) * nb)
        m = dict(base)
        m["qseq"] = qseq[sl]
        m["cseq"] = cseq[sl]
        m["cqct"] = cqct[sl]
        in_maps.append(m)
    return in_maps


def run_sharded(inputs, nb=B // NCORES, trace=False, **kw):
    if nb not in _built:
        _built[nb] = build(nb)
    nc = _built[nb]
    in_maps = make_in_maps(inputs, nb)
    res = run_bass_kernel_spmd(nc, in_maps, list(range(NCORES)), trace=trace, **kw)
    out = np.concatenate([res.results[c]["out"] for c in range(NCORES)], axis=0)
    return out.astype(np.float32), res


def kernel(**inputs):
    out, _ = run_sharded(inputs)
    return out
